# revision 2
# baseline (speedup 1.0000x reference)
"""Self-contained Trainium2 Bass kernel for nn_Attention_74474732913237.

Computation (per batch b):
  q = relu(x @ wq + bq); k = relu(y @ wk + bk); v = relu(y @ wv + bv)
  k = k @ wg
  w = softmax(mask(q @ k^T / sqrt(1024)))
  attention = w @ v
  returns (attention, w)

Strategy: pure data-parallel over batch. B=16 across 8 cores -> 2 batches per
core, no collectives. All matmuls in float32r (TF32-like, full PE rate at
N>=256, ~1.5e-4 rel err). Transposes via the PE (identity matmul).
"""

import sys

for _p in ("/root/.axon_site", "/root/.axon_site/_ro/trn_rl_repo", "/opt/trn_rl_repo"):
    if _p not in sys.path:
        sys.path.append(_p)

import numpy as np

import concourse.bacc as bacc
import concourse.bass as bass
import concourse.mybir as mybir
import concourse.tile as tile
from concourse import masks as masks_util
from concourse.bass_utils import run_bass_kernel_spmd

N_CORES = 8
BPC = 2          # batches per core
L = 2048         # LX = LY
D = 1024         # X_SIZE = Y_SIZE = ATTN
NT = L // 128    # 16 row tiles
DT = D // 128    # 8 feature tiles
F32 = mybir.dt.float32
F32R = mybir.dt.float32r
AF = mybir.ActivationFunctionType
MASK_NEG = -3.0e6   # added to masked-out scores pre-softmax-scale


def build(n_reps: int = 1):
    """Build + compile the per-core Bass program. n_reps>1 wraps the whole body
    in a hardware loop (used only for timing in test harnesses)."""
    nc = bacc.Bacc("TRN2", target_bir_lowering=False, debug=False,
                   num_devices=N_CORES)

    x_in = nc.dram_tensor("x", [BPC, L, D], F32, kind="ExternalInput").ap()
    y_in = nc.dram_tensor("y", [BPC, L, D], F32, kind="ExternalInput").ap()
    wq_in = nc.dram_tensor("wq", [D, D], F32, kind="ExternalInput").ap()
    wk_in = nc.dram_tensor("wk", [D, D], F32, kind="ExternalInput").ap()
    wv_in = nc.dram_tensor("wv", [D, D], F32, kind="ExternalInput").ap()
    wg_in = nc.dram_tensor("wg", [D, D], F32, kind="ExternalInput").ap()
    bq_in = nc.dram_tensor("bq", [D], F32, kind="ExternalInput").ap()
    bk_in = nc.dram_tensor("bk", [D], F32, kind="ExternalInput").ap()
    bv_in = nc.dram_tensor("bv", [D], F32, kind="ExternalInput").ap()
    nm_in = nc.dram_tensor("nmask", [BPC, L], F32, kind="ExternalInput").ap()

    att_out = nc.dram_tensor("att", [BPC, L, D], F32, kind="ExternalOutput").ap()
    w_out = nc.dram_tensor("w", [BPC, L, L], F32, kind="ExternalOutput").ap()

    qt_s = nc.dram_tensor("qt_s", [BPC, D, L], F32).ap()   # Q^T spill
    v_s = nc.dram_tensor("v_s", [BPC, L, D], F32).ap()     # V spill

    # weight DRAM views: (dt p) a -> p dt a   (partition = in-feature mod 128)
    wq_r = wq_in.rearrange("(dt p) a -> p dt a", p=128)
    wk_r = wk_in.rearrange("(dt p) a -> p dt a", p=128)
    wv_r = wv_in.rearrange("(dt p) a -> p dt a", p=128)
    wg_r = wg_in.rearrange("(dt p) a -> p dt a", p=128)

    with tile.TileContext(nc) as tc:
        import contextlib
        with contextlib.ExitStack() as ctx:
            bigA = ctx.enter_context(tc.tile_pool(name="bigA", bufs=1))
            bigB = ctx.enter_context(tc.tile_pool(name="bigB", bufs=1))
            wt_p = ctx.enter_context(tc.tile_pool(name="wt_p", bufs=2))
            row_p = ctx.enter_context(tc.tile_pool(name="row_p", bufs=2))
            qs_p = ctx.enter_context(tc.tile_pool(name="qs_p", bufs=2))
            u_p = ctx.enter_context(tc.tile_pool(name="u_p", bufs=2))
            wtr_p = ctx.enter_context(tc.tile_pool(name="wtr_p", bufs=1))
            st_p = ctx.enter_context(tc.tile_pool(name="st_p", bufs=3))
            sm_p = ctx.enter_context(tc.tile_pool(name="sm_p", bufs=1))
            ps_mm = ctx.enter_context(tc.tile_pool(name="ps_mm", bufs=4, space="PSUM"))
            ps_tr = ctx.enter_context(tc.tile_pool(name="ps_tr", bufs=2, space="PSUM"))
            ps_att = ctx.enter_context(tc.tile_pool(name="ps_att", bufs=2, space="PSUM"))

            # --- one-time setup ---
            ident = sm_p.tile([128, 128], F32, tag="ident")
            masks_util.make_identity(nc, ident[:])
            ones_f = sm_p.tile([1, 128], F32, tag="ones_f")
            nc.vector.memset(ones_f[:], 1.0)
            ones1 = sm_p.tile([1, 128], F32R, tag="ones1")
            nc.vector.tensor_copy(ones1[:], ones_f[:])
            bq_t = sm_p.tile([128, DT], F32, tag="bq_t")
            nc.sync.dma_start(bq_t[:], bq_in.rearrange("(t p) -> p t", p=128))
            bk_t = sm_p.tile([128, DT], F32, tag="bk_t")
            nc.sync.dma_start(bk_t[:], bk_in.rearrange("(t p) -> p t", p=128))
            bv_row = sm_p.tile([1, D], F32R, tag="bv_row")
            nc.sync.dma_start(bv_row[:], bv_in[None, :].bitcast(F32R))

            def transpose_into(dst_tiles, src_dram, bi):
                """dst_tiles: 8 tiles [128, 2048] F32R; src [L, D] natural."""
                for jt in range(NT):
                    for half in range(2):
                        row = row_p.tile([128, 512], F32, tag="row", name=f"row{bi}_{jt}_{half}")
                        nc.sync.dma_start(
                            row[:], src_dram[jt * 128:(jt + 1) * 128,
                                             half * 512:(half + 1) * 512])
                        for q in range(4):
                            dt = half * 4 + q
                            pst = ps_tr.tile([128, 128], F32, tag="tr", name=f"pst{bi}_{jt}_{dt}")
                            nc.tensor.transpose(pst[:], row[:, q * 128:(q + 1) * 128], ident[:])
                            nc.vector.tensor_copy(
                                dst_tiles[dt][:, jt * 128:(jt + 1) * 128], pst[:])

            def body():
                for bi in range(BPC):
                    # ---- A1: yT = transpose(y[bi]) -> bigA ----
                    yT = [bigA.tile([128, L], F32R, tag=f"a{i}", name=f"yT{bi}_{i}")
                          for i in range(DT)]
                    transpose_into(yT, y_in[bi], bi)

                    # ---- A2: kT = relu(wk^T @ yT + bk) -> bigB ----
                    kT = [bigB.tile([128, L], F32R, tag=f"b{i}", name=f"kT{bi}_{i}")
                          for i in range(DT)]
                    for at in range(DT):
                        wkb = wt_p.tile([128, DT, 128], F32R, tag="wt", name=f"wkb{bi}_{at}")
                        nc.sync.dma_start(
                            wkb[:], wk_r[:, :, at * 128:(at + 1) * 128].bitcast(F32R))
                        for ch in range(4):
                            psm = ps_mm.tile([128, 512], F32, tag="mm", name=f"psk{bi}_{at}_{ch}")
                            for dt in range(DT):
                                nc.tensor.matmul(
                                    psm[:], wkb[:, dt, :],
                                    yT[dt][:, ch * 512:(ch + 1) * 512],
                                    start=(dt == 0), stop=(dt == DT - 1))
                            nc.scalar.activation(
                                kT[at][:, ch * 512:(ch + 1) * 512], psm[:],
                                AF.Relu, bias=bk_t[:, at:at + 1])

                    # ---- A3: V = relu(y @ wv + bv) -> v_s (DRAM) ----
                    for ach in range(4):  # a-chunks of 256
                        wvb = wt_p.tile([128, DT, 256], F32R, tag="wt", name=f"wvb{bi}_{ach}")
                        nc.sync.dma_start(
                            wvb[:], wv_r[:, :, ach * 256:(ach + 1) * 256].bitcast(F32R))
                        for jt in range(NT):
                            psv = ps_mm.tile([128, 256], F32, tag="mm", name=f"psv{bi}_{ach}_{jt}")
                            for dt in range(DT):
                                nc.tensor.matmul(
                                    psv[:], yT[dt][:, jt * 128:(jt + 1) * 128],
                                    wvb[:, dt, :],
                                    start=(dt == 0), stop=False)
                            nc.tensor.matmul(
                                psv[:], ones1[:],
                                bv_row[:, ach * 256:(ach + 1) * 256],
                                start=False, stop=True)
                            vst = st_p.tile([128, 512], F32, tag="st", name=f"vst{bi}_{ach}_{jt}")
                            nc.scalar.activation(vst[:, :256], psv[:], AF.Relu)
                            nc.sync.dma_start(
                                v_s[bi, jt * 128:(jt + 1) * 128,
                                    ach * 256:(ach + 1) * 256], vst[:, :256])

                    # ---- A4: kgT = wg^T @ kT -> bigA (reuse yT slots) ----
                    kgT = [bigA.tile([128, L], F32R, tag=f"a{i}", name=f"kgT{bi}_{i}")
                           for i in range(DT)]
                    for at2 in range(DT):
                        wgb = wt_p.tile([128, DT, 128], F32R, tag="wt", name=f"wgb{bi}_{at2}")
                        nc.sync.dma_start(
                            wgb[:], wg_r[:, :, at2 * 128:(at2 + 1) * 128].bitcast(F32R))
                        for ch in range(4):
                            psg = ps_mm.tile([128, 512], F32, tag="mm", name=f"psg{bi}_{at2}_{ch}")
                            for at in range(DT):
                                nc.tensor.matmul(
                                    psg[:], wgb[:, at, :],
                                    kT[at][:, ch * 512:(ch + 1) * 512],
                                    start=(at == 0), stop=(at == DT - 1))
                            nc.vector.tensor_copy(
                                kgT[at2][:, ch * 512:(ch + 1) * 512], psg[:])

                    # ---- A5: xT = transpose(x[bi]) -> bigB (reuse kT slots) ----
                    xT = [bigB.tile([128, L], F32R, tag=f"b{i}", name=f"xT{bi}_{i}")
                          for i in range(DT)]
                    transpose_into(xT, x_in[bi], BPC + bi)

                    # ---- A6: Q^T = relu(wq^T @ xT + bq) -> qt_s (DRAM) ----
                    for at in range(DT):
                        wqb = wt_p.tile([128, DT, 128], F32R, tag="wt", name=f"wqb{bi}_{at}")
                        nc.sync.dma_start(
                            wqb[:], wq_r[:, :, at * 128:(at + 1) * 128].bitcast(F32R))
                        for ch in range(4):
                            psq = ps_mm.tile([128, 512], F32, tag="mm", name=f"psq{bi}_{at}_{ch}")
                            for dt in range(DT):
                                nc.tensor.matmul(
                                    psq[:], wqb[:, dt, :],
                                    xT[dt][:, ch * 512:(ch + 1) * 512],
                                    start=(dt == 0), stop=(dt == DT - 1))
                            qst = st_p.tile([128, 512], F32, tag="st", name=f"qst{bi}_{at}_{ch}")
                            nc.scalar.activation(qst[:], psq[:], AF.Relu,
                                                 bias=bq_t[:, at:at + 1])
                            nc.sync.dma_start(
                                qt_s[bi, at * 128:(at + 1) * 128,
                                     ch * 512:(ch + 1) * 512], qst[:])

                    # ---- B: attention ----
                    vB = [bigB.tile([128, 2, D], F32R, tag=f"b{i}", name=f"vB{bi}_{i}")
                          for i in range(DT)]
                    for i in range(DT):
                        for s in range(2):
                            jt = 2 * i + s
                            nc.sync.dma_start(
                                vB[i][:, s, :],
                                v_s[bi, jt * 128:(jt + 1) * 128, :].bitcast(F32R))
                    nmr = sm_p.tile([1, L], F32R, tag="nm", bufs=2, name=f"nmr{bi}")
                    nc.sync.dma_start(nmr[:], nm_in[bi:bi + 1, :].bitcast(F32R))

                    qt_r = qt_s[bi].rearrange("(at p) i -> p at i", p=128)
                    for isub in range(NT):
                        qs = qs_p.tile([128, DT, 128], F32R, tag="qs", name=f"qs{bi}_{isub}")
                        nc.sync.dma_start(
                            qs[:], qt_r[:, :, isub * 128:(isub + 1) * 128].bitcast(F32R))
                        U = u_p.tile([128, L], F32, tag="u", name=f"U{bi}_{isub}")
                        rs4 = sm_p.tile([128, 4], F32, tag="rs4", bufs=2, name=f"rs4{bi}_{isub}")
                        for ch in range(4):
                            pss = ps_mm.tile([128, 512], F32, tag="mm", name=f"pss{bi}_{isub}_{ch}")
                            for at in range(DT):
                                nc.tensor.matmul(
                                    pss[:], qs[:, at, :],
                                    kgT[at][:, ch * 512:(ch + 1) * 512],
                                    start=(at == 0), stop=False)
                            nc.tensor.matmul(
                                pss[:], ones1[:], nmr[:, ch * 512:(ch + 1) * 512],
                                start=False, stop=True)
                            nc.scalar.activation(
                                U[:, ch * 512:(ch + 1) * 512], pss[:], AF.Exp,
                                scale=1.0 / 32.0, accum_out=rs4[:, ch:ch + 1])
                        rs1 = sm_p.tile([128, 1], F32, tag="rs1", bufs=2, name=f"rs1{bi}_{isub}")
                        nc.vector.reduce_sum(rs1[:], rs4[:], axis=mybir.AxisListType.X)
                        rcp = sm_p.tile([128, 1], F32, tag="rcp", bufs=2, name=f"rcp{bi}_{isub}")
                        nc.vector.reciprocal(rcp[:], rs1[:])

                        # transpose RAW U (PE), normalize U in place afterwards
                        WT = wtr_p.tile([128, NT, 128], F32R, tag="wtr", name=f"WT{bi}_{isub}")
                        for jt in range(NT):
                            pst = ps_tr.tile([128, 128], F32, tag="tr", name=f"pstw{bi}_{isub}_{jt}")
                            nc.tensor.transpose(
                                pst[:], U[:, jt * 128:(jt + 1) * 128], ident[:])
                            nc.vector.tensor_copy(WT[:, jt, :], pst[:])
                        nc.vector.tensor_scalar_mul(U[:], U[:], rcp[:])
                        nc.sync.dma_start(w_out[bi, isub * 128:(isub + 1) * 128, :], U[:])

                        for ach in range(2):
                            psa = ps_att.tile([128, 512], F32, tag="att", name=f"psa{bi}_{isub}_{ach}")
                            for jt in range(NT):
                                nc.tensor.matmul(
                                    psa[:], WT[:, jt, :],
                                    vB[jt // 2][:, jt % 2, ach * 512:(ach + 1) * 512],
                                    start=(jt == 0), stop=(jt == NT - 1))
                            ast = st_p.tile([128, 512], F32, tag="st", name=f"ast{bi}_{isub}_{ach}")
                            # attention normalization folded into the copy
                            nc.scalar.activation(ast[:], psa[:], AF.Copy, scale=rcp[:])
                            nc.sync.dma_start(
                                att_out[bi, isub * 128:(isub + 1) * 128,
                                        ach * 512:(ach + 1) * 512], ast[:])

            if n_reps == 1:
                body()
            else:
                with tc.For_i(0, n_reps, 1):
                    body()

    nc.compile()
    return nc


_cache = {}


def _get_nc(n_reps: int = 1):
    if n_reps not in _cache:
        _cache[n_reps] = build(n_reps)
    return _cache[n_reps]


def make_in_maps(x, y, wq, bq, wk, bk, wv, bv, wg, masks):
    x = np.ascontiguousarray(np.asarray(x, dtype=np.float32))
    y = np.ascontiguousarray(np.asarray(y, dtype=np.float32))
    nmask = np.where(np.asarray(masks), np.float32(0), np.float32(MASK_NEG))
    nmask = np.ascontiguousarray(nmask.astype(np.float32))
    ws = {n: np.ascontiguousarray(np.asarray(a, dtype=np.float32))
          for n, a in (("wq", wq), ("wk", wk), ("wv", wv), ("wg", wg),
                       ("bq", bq), ("bk", bk), ("bv", bv))}
    in_maps = []
    for c in range(N_CORES):
        sl = slice(c * BPC, (c + 1) * BPC)
        in_maps.append({
            "x": np.ascontiguousarray(x[sl]),
            "y": np.ascontiguousarray(y[sl]),
            "nmask": np.ascontiguousarray(nmask[sl]),
            **ws,
        })
    return in_maps


def kernel(x, y, wq, bq, wk, bk, wv, bv, wg, masks):
    nc = _get_nc(1)
    in_maps = make_in_maps(x, y, wq, bq, wk, bk, wv, bv, wg, masks)
    res = run_bass_kernel_spmd(nc, in_maps, list(range(N_CORES)))
    attention = np.concatenate([res.results[c]["att"] for c in range(N_CORES)], axis=0)
    w = np.concatenate([res.results[c]["w"] for c in range(N_CORES)], axis=0)
    return (attention, w)


# revision 8
# speedup vs baseline: 1.2435x; 1.2435x over previous
"""Self-contained Trainium2 Bass kernel for nn_Attention_74474732913237.

Computation (per batch b):
  q = relu(x @ wq + bq); k = relu(y @ wk + bk); v = relu(y @ wv + bv)
  k = k @ wg
  w = softmax(mask(q @ k^T / sqrt(1024)))
  attention = w @ v
  returns (attention, w)

Strategy: pure data-parallel over batch. B=16 across 8 cores -> 2 batches per
core, no collectives. All matmuls in float32r (TF32-like, full PE rate at
N>=256, ~1.5e-4 rel err). Transposes via the PE (identity matmul).
"""

import sys

for _p in ("/root/.axon_site", "/root/.axon_site/_ro/trn_rl_repo", "/opt/trn_rl_repo"):
    if _p not in sys.path:
        sys.path.append(_p)

import numpy as np

import concourse.bacc as bacc
import concourse.bass as bass
import concourse.mybir as mybir
import concourse.tile as tile
from concourse import masks as masks_util
from concourse.bass_utils import run_bass_kernel_spmd

N_CORES = 8
BPC = 2          # batches per core
L = 2048         # LX = LY
D = 1024         # X_SIZE = Y_SIZE = ATTN
NT = L // 128    # 16 row tiles
DT = D // 128    # 8 feature tiles
F32 = mybir.dt.float32
F32R = mybir.dt.float32r
AF = mybir.ActivationFunctionType
MASK_NEG = -3.0e6   # added to masked-out scores pre-softmax-scale


def build(n_reps: int = 1, phase_limit: int = 99):
    """Build + compile the per-core Bass program. n_reps>1 wraps the whole body
    in a hardware loop; phase_limit truncates phases (both for timing only)."""
    nc = bacc.Bacc("TRN2", target_bir_lowering=False, debug=False,
                   num_devices=N_CORES)

    x_in = nc.dram_tensor("x", [BPC, L, D], F32, kind="ExternalInput").ap()
    y_in = nc.dram_tensor("y", [BPC, L, D], F32, kind="ExternalInput").ap()
    # host-preformatted weights: lhsT-style [at, p, dt, 128]; wv [ach, p, dt, 256]
    wq_in = nc.dram_tensor("wqp", [DT, 128, DT, 128], F32, kind="ExternalInput").ap()
    wk_in = nc.dram_tensor("wkp", [DT, 128, DT, 128], F32, kind="ExternalInput").ap()
    wv_in = nc.dram_tensor("wvp", [4, 128, DT, 256], F32, kind="ExternalInput").ap()
    wg_in = nc.dram_tensor("wgp", [DT, 128, DT, 128], F32, kind="ExternalInput").ap()
    bq_in = nc.dram_tensor("bq", [D], F32, kind="ExternalInput").ap()
    bk_in = nc.dram_tensor("bk", [D], F32, kind="ExternalInput").ap()
    bv_in = nc.dram_tensor("bv", [D], F32, kind="ExternalInput").ap()
    nm_in = nc.dram_tensor("nmask", [BPC, L], F32, kind="ExternalInput").ap()

    att_out = nc.dram_tensor("att", [BPC, L, D], F32, kind="ExternalOutput").ap()
    w_out = nc.dram_tensor("w", [BPC, L, L], F32, kind="ExternalOutput").ap()

    qt_s = nc.dram_tensor("qt_s", [BPC, D, L], F32).ap()   # Q^T spill
    v_s = nc.dram_tensor("v_s", [BPC, L, D], F32).ap()     # V spill


    with tile.TileContext(nc) as tc:
        import contextlib
        with contextlib.ExitStack() as ctx:
            bigA = ctx.enter_context(tc.tile_pool(name="bigA", bufs=1))
            bigB = ctx.enter_context(tc.tile_pool(name="bigB", bufs=1))
            wt_p = ctx.enter_context(tc.tile_pool(name="wt_p", bufs=2))
            row_p = ctx.enter_context(tc.tile_pool(name="row_p", bufs=2))
            qs_p = ctx.enter_context(tc.tile_pool(name="qs_p", bufs=3))
            u_p = ctx.enter_context(tc.tile_pool(name="u_p", bufs=2))
            wtr_p = ctx.enter_context(tc.tile_pool(name="wtr_p", bufs=1))
            st_p = ctx.enter_context(tc.tile_pool(name="st_p", bufs=3))
            sm_p = ctx.enter_context(tc.tile_pool(name="sm_p", bufs=1))
            ps_mm = ctx.enter_context(tc.tile_pool(name="ps_mm", bufs=4, space="PSUM"))
            ps_tr = ctx.enter_context(tc.tile_pool(name="ps_tr", bufs=2, space="PSUM"))
            ps_att = ctx.enter_context(tc.tile_pool(name="ps_att", bufs=2, space="PSUM"))

            # --- one-time setup ---
            ident = sm_p.tile([128, 128], F32, tag="ident")
            masks_util.make_identity(nc, ident[:])
            ones_f = sm_p.tile([1, 128], F32, tag="ones_f")
            nc.vector.memset(ones_f[:], 1.0)
            ones1 = sm_p.tile([1, 128], F32R, tag="ones1")
            nc.vector.tensor_copy(ones1[:], ones_f[:])
            bq_t = sm_p.tile([128, DT], F32, tag="bq_t")
            nc.sync.dma_start(bq_t[:], bq_in.rearrange("(t p) -> p t", p=128))
            bk_t = sm_p.tile([128, DT], F32, tag="bk_t")
            nc.sync.dma_start(bk_t[:], bk_in.rearrange("(t p) -> p t", p=128))
            bv_row = sm_p.tile([1, D], F32R, tag="bv_row")
            nc.sync.dma_start(bv_row[:], bv_in[None, :].bitcast(F32R))

            def transpose_into(dst_tiles, src_dram, bi):
                """dst_tiles: 8 tiles [128, 2048] F32R; src [L, D] natural."""
                for jt in range(NT):
                    for half in range(2):
                        row = row_p.tile([128, 512], F32, tag="row", name=f"row{bi}_{jt}_{half}")
                        nc.sync.dma_start(
                            row[:], src_dram[jt * 128:(jt + 1) * 128,
                                             half * 512:(half + 1) * 512])
                        for q in range(4):
                            dt = half * 4 + q
                            pst = ps_tr.tile([128, 128], F32, tag="tr", name=f"pst{bi}_{jt}_{dt}")
                            nc.tensor.transpose(pst[:], row[:, q * 128:(q + 1) * 128], ident[:])
                            dst = dst_tiles[dt][:, jt * 128:(jt + 1) * 128]
                            if q % 2 == 0:
                                nc.vector.tensor_copy(dst, pst[:])
                            else:
                                nc.scalar.copy(dst, pst[:])

            def body():
                for bi in range(BPC):
                    # ---- A1: yT = transpose(y[bi]) -> bigA ----
                    yT = [bigA.tile([128, L], F32R, tag=f"a{i}", name=f"yT{bi}_{i}")
                          for i in range(DT)]
                    transpose_into(yT, y_in[bi], bi)
                    if phase_limit < 2:
                        continue

                    # ---- A2: kT = relu(wk^T @ yT + bk) -> bigB ----
                    kT = [bigB.tile([128, L], F32R, tag=f"b{i}", name=f"kT{bi}_{i}")
                          for i in range(DT)]
                    for at in range(DT):
                        wkb = wt_p.tile([128, DT, 128], F32R, tag="wt", name=f"wkb{bi}_{at}")
                        nc.sync.dma_start(wkb[:], wk_in[at].bitcast(F32R))
                        for ch in range(4):
                            psm = ps_mm.tile([128, 512], F32, tag="mm", name=f"psk{bi}_{at}_{ch}")
                            for dt in range(DT):
                                nc.tensor.matmul(
                                    psm[:], wkb[:, dt, :],
                                    yT[dt][:, ch * 512:(ch + 1) * 512],
                                    start=(dt == 0), stop=(dt == DT - 1))
                            nc.scalar.activation(
                                kT[at][:, ch * 512:(ch + 1) * 512], psm[:],
                                AF.Relu, bias=bk_t[:, at:at + 1])

                    if phase_limit < 3:
                        continue
                    # ---- A3: V = relu(y @ wv + bv) -> v_s (DRAM) ----
                    for ach in range(4):  # a-chunks of 256
                        wvb = wt_p.tile([128, DT, 256], F32R, tag="wt", name=f"wvb{bi}_{ach}")
                        nc.sync.dma_start(wvb[:], wv_in[ach].bitcast(F32R))
                        for jt in range(NT):
                            psv = ps_mm.tile([128, 256], F32, tag="mm", name=f"psv{bi}_{ach}_{jt}")
                            for dt in range(DT):
                                nc.tensor.matmul(
                                    psv[:], yT[dt][:, jt * 128:(jt + 1) * 128],
                                    wvb[:, dt, :],
                                    start=(dt == 0), stop=False)
                            nc.tensor.matmul(
                                psv[:], ones1[:],
                                bv_row[:, ach * 256:(ach + 1) * 256],
                                start=False, stop=True)
                            vst = st_p.tile([128, 512], F32, tag="st", name=f"vst{bi}_{ach}_{jt}")
                            nc.scalar.activation(vst[:, :256], psv[:], AF.Relu)
                            nc.sync.dma_start(
                                v_s[bi, jt * 128:(jt + 1) * 128,
                                    ach * 256:(ach + 1) * 256], vst[:, :256])

                    if phase_limit < 4:
                        continue
                    # ---- A4: kgT = wg^T @ kT -> bigA (reuse yT slots) ----
                    kgT = [bigA.tile([128, L], F32R, tag=f"a{i}", name=f"kgT{bi}_{i}")
                           for i in range(DT)]
                    for at2 in range(DT):
                        wgb = wt_p.tile([128, DT, 128], F32R, tag="wt", name=f"wgb{bi}_{at2}")
                        nc.sync.dma_start(wgb[:], wg_in[at2].bitcast(F32R))
                        for ch in range(4):
                            psg = ps_mm.tile([128, 512], F32, tag="mm", name=f"psg{bi}_{at2}_{ch}")
                            for at in range(DT):
                                nc.tensor.matmul(
                                    psg[:], wgb[:, at, :],
                                    kT[at][:, ch * 512:(ch + 1) * 512],
                                    start=(at == 0), stop=(at == DT - 1))
                            nc.vector.tensor_copy(
                                kgT[at2][:, ch * 512:(ch + 1) * 512], psg[:])

                    if phase_limit < 5:
                        continue
                    # ---- A5: xT = transpose(x[bi]) -> bigB (reuse kT slots) ----
                    xT = [bigB.tile([128, L], F32R, tag=f"b{i}", name=f"xT{bi}_{i}")
                          for i in range(DT)]
                    transpose_into(xT, x_in[bi], BPC + bi)

                    # ---- A6: Q^T = relu(wq^T @ xT + bq) -> qt_s (DRAM) ----
                    for at in range(DT):
                        wqb = wt_p.tile([128, DT, 128], F32R, tag="wt", name=f"wqb{bi}_{at}")
                        nc.sync.dma_start(wqb[:], wq_in[at].bitcast(F32R))
                        for ch in range(4):
                            psq = ps_mm.tile([128, 512], F32, tag="mm", name=f"psq{bi}_{at}_{ch}")
                            for dt in range(DT):
                                nc.tensor.matmul(
                                    psq[:], wqb[:, dt, :],
                                    xT[dt][:, ch * 512:(ch + 1) * 512],
                                    start=(dt == 0), stop=(dt == DT - 1))
                            qst = st_p.tile([128, 512], F32, tag="st", name=f"qst{bi}_{at}_{ch}")
                            nc.scalar.activation(qst[:], psq[:], AF.Relu,
                                                 bias=bq_t[:, at:at + 1])
                            nc.sync.dma_start(
                                qt_s[bi, at * 128:(at + 1) * 128,
                                     ch * 512:(ch + 1) * 512], qst[:])

                    if phase_limit < 6:
                        continue
                    # ---- B: attention ----
                    vB = [bigB.tile([128, 2, D], F32R, tag=f"b{i}", name=f"vB{bi}_{i}")
                          for i in range(DT)]
                    for i in range(DT):
                        for s in range(2):
                            jt = 2 * i + s
                            nc.sync.dma_start(
                                vB[i][:, s, :],
                                v_s[bi, jt * 128:(jt + 1) * 128, :].bitcast(F32R))
                    nmr = sm_p.tile([1, L], F32R, tag="nm", bufs=1, name=f"nmr{bi}")
                    nc.sync.dma_start(nmr[:], nm_in[bi:bi + 1, :].bitcast(F32R))

                    qt_r = qt_s[bi].rearrange("(at p) i -> p at i", p=128)
                    for isub in range(NT):
                        qs = qs_p.tile([128, DT, 128], F32R, tag="qs", name=f"qs{bi}_{isub}")
                        nc.sync.dma_start(
                            qs[:], qt_r[:, :, isub * 128:(isub + 1) * 128].bitcast(F32R))
                        U = u_p.tile([128, L], F32, tag="u", name=f"U{bi}_{isub}")
                        rs4 = sm_p.tile([128, 4], F32, tag="rs4", bufs=2, name=f"rs4{bi}_{isub}")
                        for ch in range(4):
                            pss = ps_mm.tile([128, 512], F32, tag="mm", name=f"pss{bi}_{isub}_{ch}")
                            for at in range(DT):
                                nc.tensor.matmul(
                                    pss[:], qs[:, at, :],
                                    kgT[at][:, ch * 512:(ch + 1) * 512],
                                    start=(at == 0), stop=False)
                            nc.tensor.matmul(
                                pss[:], ones1[:], nmr[:, ch * 512:(ch + 1) * 512],
                                start=False, stop=True)
                            nc.scalar.activation(
                                U[:, ch * 512:(ch + 1) * 512], pss[:], AF.Exp,
                                scale=1.0 / 32.0, accum_out=rs4[:, ch:ch + 1])
                        rs1 = sm_p.tile([128, 1], F32, tag="rs1", bufs=2, name=f"rs1{bi}_{isub}")
                        nc.vector.reduce_sum(rs1[:], rs4[:], axis=mybir.AxisListType.X)
                        rcp = sm_p.tile([128, 1], F32, tag="rcp", bufs=2, name=f"rcp{bi}_{isub}")
                        nc.vector.reciprocal(rcp[:], rs1[:])

                        # transpose RAW U (PE), normalize U in place afterwards
                        if phase_limit < 7:
                            nc.vector.tensor_scalar_mul(U[:], U[:], rcp[:])
                            nc.sync.dma_start(w_out[bi, isub * 128:(isub + 1) * 128, :], U[:])
                            continue
                        WT = wtr_p.tile([128, NT, 128], F32R, tag="wtr", name=f"WT{bi}_{isub}")
                        for jt in range(NT):
                            pst = ps_tr.tile([128, 128], F32, tag="tr", name=f"pstw{bi}_{isub}_{jt}")
                            nc.tensor.transpose(
                                pst[:], U[:, jt * 128:(jt + 1) * 128], ident[:])
                            if jt % 2 == 0:
                                nc.vector.tensor_copy(WT[:, jt, :], pst[:])
                            else:
                                nc.scalar.copy(WT[:, jt, :], pst[:])
                        nc.vector.tensor_scalar_mul(U[:], U[:], rcp[:])
                        nc.sync.dma_start(w_out[bi, isub * 128:(isub + 1) * 128, :], U[:])

                        for ach in range(2):
                            psa = ps_att.tile([128, 512], F32, tag="att", name=f"psa{bi}_{isub}_{ach}")
                            for jt in range(NT):
                                nc.tensor.matmul(
                                    psa[:], WT[:, jt, :],
                                    vB[jt // 2][:, jt % 2, ach * 512:(ach + 1) * 512],
                                    start=(jt == 0), stop=(jt == NT - 1))
                            ast = st_p.tile([128, 512], F32, tag="st", name=f"ast{bi}_{isub}_{ach}")
                            # attention normalization folded into the copy
                            nc.scalar.activation(ast[:], psa[:], AF.Copy, scale=rcp[:])
                            nc.sync.dma_start(
                                att_out[bi, isub * 128:(isub + 1) * 128,
                                        ach * 512:(ach + 1) * 512], ast[:])

            if n_reps == 1:
                body()
            else:
                with tc.For_i(0, n_reps, 1):
                    body()

    nc.compile()
    return nc


_cache = {}


def _get_nc(n_reps: int = 1):
    if n_reps not in _cache:
        _cache[n_reps] = build(n_reps)
    return _cache[n_reps]


def _fmt_lhs(w):
    w4 = np.asarray(w, np.float32).reshape(DT, 128, DT, 128)   # (dt, p, at, c)
    return np.ascontiguousarray(w4.transpose(2, 1, 0, 3))      # (at, p, dt, c)


def _fmt_mov(w):
    w4 = np.asarray(w, np.float32).reshape(DT, 128, 4, 256)    # (dt, p, ach, c)
    return np.ascontiguousarray(w4.transpose(2, 1, 0, 3))      # (ach, p, dt, c)


def make_in_maps(x, y, wq, bq, wk, bk, wv, bv, wg, masks):
    x = np.ascontiguousarray(np.asarray(x, dtype=np.float32))
    y = np.ascontiguousarray(np.asarray(y, dtype=np.float32))
    nmask = np.where(np.asarray(masks), np.float32(0), np.float32(MASK_NEG))
    nmask = np.ascontiguousarray(nmask.astype(np.float32))
    ws = {"wqp": _fmt_lhs(wq), "wkp": _fmt_lhs(wk), "wvp": _fmt_mov(wv),
          "wgp": _fmt_lhs(wg)}
    ws.update({n: np.ascontiguousarray(np.asarray(a, dtype=np.float32))
               for n, a in (("bq", bq), ("bk", bk), ("bv", bv))})
    in_maps = []
    for c in range(N_CORES):
        sl = slice(c * BPC, (c + 1) * BPC)
        in_maps.append({
            "x": np.ascontiguousarray(x[sl]),
            "y": np.ascontiguousarray(y[sl]),
            "nmask": np.ascontiguousarray(nmask[sl]),
            **ws,
        })
    return in_maps


def kernel(x, y, wq, bq, wk, bk, wv, bv, wg, masks):
    nc = _get_nc(1)
    in_maps = make_in_maps(x, y, wq, bq, wk, bk, wv, bv, wg, masks)
    res = run_bass_kernel_spmd(nc, in_maps, list(range(N_CORES)))
    attention = np.concatenate([res.results[c]["att"] for c in range(N_CORES)], axis=0)
    w = np.concatenate([res.results[c]["w"] for c in range(N_CORES)], axis=0)
    return (attention, w)


# revision 10
# speedup vs baseline: 107.9588x; 86.8220x over previous
"""Self-contained Trainium2 Bass kernel for nn_Attention_74474732913237.

Computation (per batch b):
  q = relu(x @ wq + bq); k = relu(y @ wk + bk); v = relu(y @ wv + bv)
  k = k @ wg
  w = softmax(mask(q @ k^T / sqrt(1024)))
  attention = w @ v
  returns (attention, w)

Strategy: pure data-parallel over batch. B=16 across 8 cores -> 2 batches per
core, no collectives. All matmuls in float32r (TF32-like, full PE rate at
N>=256, ~1.5e-4 rel err). Transposes via the PE (identity matmul).
"""

import sys

for _p in ("/root/.axon_site", "/root/.axon_site/_ro/trn_rl_repo", "/opt/trn_rl_repo"):
    if _p not in sys.path:
        sys.path.append(_p)

import numpy as np

import concourse.bacc as bacc
import concourse.bass as bass
import concourse.mybir as mybir
import concourse.tile as tile
from concourse import masks as masks_util
from concourse.bass_utils import run_bass_kernel_spmd

N_CORES = 8
BPC = 2          # batches per core
L = 2048         # LX = LY
D = 1024         # X_SIZE = Y_SIZE = ATTN
NT = L // 128    # 16 row tiles
DT = D // 128    # 8 feature tiles
F32 = mybir.dt.float32
F32R = mybir.dt.float32r
AF = mybir.ActivationFunctionType
MASK_NEG = -3.0e6   # added to masked-out scores pre-softmax-scale


def build(n_reps: int = 1, phase_limit: int = 99):
    """Build + compile the per-core Bass program. n_reps>1 wraps the whole body
    in a hardware loop; phase_limit truncates phases (both for timing only)."""
    nc = bacc.Bacc("TRN2", target_bir_lowering=False, debug=False,
                   num_devices=N_CORES)

    x_in = nc.dram_tensor("x", [BPC, L, D], F32, kind="ExternalInput").ap()
    y_in = nc.dram_tensor("y", [BPC, L, D], F32, kind="ExternalInput").ap()
    # host-preformatted weights: lhsT-style [at, p, dt, 128]; wv [ach, p, dt, 256]
    wq_in = nc.dram_tensor("wqp", [DT, 128, DT, 128], F32, kind="ExternalInput").ap()
    wk_in = nc.dram_tensor("wkp", [DT, 128, DT, 128], F32, kind="ExternalInput").ap()
    wv_in = nc.dram_tensor("wvp", [4, 128, DT, 256], F32, kind="ExternalInput").ap()
    wg_in = nc.dram_tensor("wgp", [DT, 128, DT, 128], F32, kind="ExternalInput").ap()
    bq_in = nc.dram_tensor("bq", [D], F32, kind="ExternalInput").ap()
    bk_in = nc.dram_tensor("bk", [D], F32, kind="ExternalInput").ap()
    bv_in = nc.dram_tensor("bv", [D], F32, kind="ExternalInput").ap()
    nm_in = nc.dram_tensor("nmask", [BPC, L], F32, kind="ExternalInput").ap()

    att_out = nc.dram_tensor("att", [BPC, L, D], F32, kind="ExternalOutput").ap()
    w_out = nc.dram_tensor("w", [BPC, L, L], F32, kind="ExternalOutput").ap()

    qt_s = nc.dram_tensor("qt_s", [BPC, D, L], F32).ap()   # Q^T spill
    v_s = nc.dram_tensor("v_s", [BPC, L, D], F32).ap()     # V spill


    with tile.TileContext(nc) as tc:
        import contextlib
        with contextlib.ExitStack() as ctx:
            bigA = ctx.enter_context(tc.tile_pool(name="bigA", bufs=1))
            bigB = ctx.enter_context(tc.tile_pool(name="bigB", bufs=1))
            wt_p = ctx.enter_context(tc.tile_pool(name="wt_p", bufs=2))
            row_p = ctx.enter_context(tc.tile_pool(name="row_p", bufs=2))
            qs_p = ctx.enter_context(tc.tile_pool(name="qs_p", bufs=3))
            u_p = ctx.enter_context(tc.tile_pool(name="u_p", bufs=2))
            wtr_p = ctx.enter_context(tc.tile_pool(name="wtr_p", bufs=1))
            st_p = ctx.enter_context(tc.tile_pool(name="st_p", bufs=3))
            sm_p = ctx.enter_context(tc.tile_pool(name="sm_p", bufs=1))
            ps_mm = ctx.enter_context(tc.tile_pool(name="ps_mm", bufs=4, space="PSUM"))
            ps_tr = ctx.enter_context(tc.tile_pool(name="ps_tr", bufs=2, space="PSUM"))
            ps_att = ctx.enter_context(tc.tile_pool(name="ps_att", bufs=2, space="PSUM"))

            # --- one-time setup ---
            ident = sm_p.tile([128, 128], F32, tag="ident")
            masks_util.make_identity(nc, ident[:])
            ones_f = sm_p.tile([1, 128], F32, tag="ones_f")
            nc.vector.memset(ones_f[:], 1.0)
            ones1 = sm_p.tile([1, 128], F32R, tag="ones1")
            nc.vector.tensor_copy(ones1[:], ones_f[:])
            bq_t = sm_p.tile([128, DT], F32, tag="bq_t")
            nc.sync.dma_start(bq_t[:], bq_in.rearrange("(t p) -> p t", p=128))
            bk_t = sm_p.tile([128, DT], F32, tag="bk_t")
            nc.sync.dma_start(bk_t[:], bk_in.rearrange("(t p) -> p t", p=128))
            bv_row = sm_p.tile([1, D], F32R, tag="bv_row")
            nc.sync.dma_start(bv_row[:], bv_in[None, :].bitcast(F32R))

            def transpose_into(dst_tiles, src_dram, bi):
                """dst_tiles: 8 tiles [128, 2048] F32R; src [L, D] natural."""
                for jt in range(NT):
                    for half in range(2):
                        row = row_p.tile([128, 512], F32, tag="row", name=f"row{bi}_{jt}_{half}")
                        nc.sync.dma_start(
                            row[:], src_dram[jt * 128:(jt + 1) * 128,
                                             half * 512:(half + 1) * 512])
                        for q in range(4):
                            dt = half * 4 + q
                            pst = ps_tr.tile([128, 128], F32, tag="tr", name=f"pst{bi}_{jt}_{dt}")
                            nc.tensor.transpose(pst[:], row[:, q * 128:(q + 1) * 128], ident[:])
                            dst = dst_tiles[dt][:, jt * 128:(jt + 1) * 128]
                            if q % 2 == 0:
                                nc.vector.tensor_copy(dst, pst[:])
                            else:
                                nc.scalar.copy(dst, pst[:])

            def body():
                for bi in range(BPC):
                    # ---- A1: yT = transpose(y[bi]) -> bigA ----
                    yT = [bigA.tile([128, L], F32R, tag=f"a{i}", name=f"yT{bi}_{i}")
                          for i in range(DT)]
                    transpose_into(yT, y_in[bi], bi)
                    if phase_limit < 2:
                        continue

                    # ---- A2: kT = relu(wk^T @ yT + bk) -> bigB ----
                    kT = [bigB.tile([128, L], F32R, tag=f"b{i}", name=f"kT{bi}_{i}")
                          for i in range(DT)]
                    for at in range(DT):
                        wkb = wt_p.tile([128, DT, 128], F32R, tag="wt", name=f"wkb{bi}_{at}")
                        nc.sync.dma_start(wkb[:], wk_in[at].bitcast(F32R))
                        for ch in range(4):
                            psm = ps_mm.tile([128, 512], F32, tag="mm", name=f"psk{bi}_{at}_{ch}")
                            for dt in range(DT):
                                nc.tensor.matmul(
                                    psm[:], wkb[:, dt, :],
                                    yT[dt][:, ch * 512:(ch + 1) * 512],
                                    start=(dt == 0), stop=(dt == DT - 1))
                            nc.scalar.activation(
                                kT[at][:, ch * 512:(ch + 1) * 512], psm[:],
                                AF.Relu, bias=bk_t[:, at:at + 1])

                    if phase_limit < 3:
                        continue
                    # ---- A3: V = relu(y @ wv + bv) -> v_s (DRAM) ----
                    for ach in range(4):  # a-chunks of 256
                        wvb = wt_p.tile([128, DT, 256], F32R, tag="wt", name=f"wvb{bi}_{ach}")
                        nc.sync.dma_start(wvb[:], wv_in[ach].bitcast(F32R))
                        for jt in range(NT):
                            psv = ps_mm.tile([128, 256], F32, tag="mm", name=f"psv{bi}_{ach}_{jt}")
                            for dt in range(DT):
                                nc.tensor.matmul(
                                    psv[:], yT[dt][:, jt * 128:(jt + 1) * 128],
                                    wvb[:, dt, :],
                                    start=(dt == 0), stop=False)
                            nc.tensor.matmul(
                                psv[:], ones1[:],
                                bv_row[:, ach * 256:(ach + 1) * 256],
                                start=False, stop=True)
                            vst = st_p.tile([128, 512], F32, tag="st", name=f"vst{bi}_{ach}_{jt}")
                            nc.scalar.activation(vst[:, :256], psv[:], AF.Relu)
                            nc.sync.dma_start(
                                v_s[bi, jt * 128:(jt + 1) * 128,
                                    ach * 256:(ach + 1) * 256], vst[:, :256])

                    if phase_limit < 4:
                        continue
                    # ---- A4: kgT = wg^T @ kT -> bigA (reuse yT slots) ----
                    kgT = [bigA.tile([128, L], F32R, tag=f"a{i}", name=f"kgT{bi}_{i}")
                           for i in range(DT)]
                    for at2 in range(DT):
                        wgb = wt_p.tile([128, DT, 128], F32R, tag="wt", name=f"wgb{bi}_{at2}")
                        nc.sync.dma_start(wgb[:], wg_in[at2].bitcast(F32R))
                        for ch in range(4):
                            psg = ps_mm.tile([128, 512], F32, tag="mm", name=f"psg{bi}_{at2}_{ch}")
                            for at in range(DT):
                                nc.tensor.matmul(
                                    psg[:], wgb[:, at, :],
                                    kT[at][:, ch * 512:(ch + 1) * 512],
                                    start=(at == 0), stop=(at == DT - 1))
                            nc.vector.tensor_copy(
                                kgT[at2][:, ch * 512:(ch + 1) * 512], psg[:])

                    if phase_limit < 5:
                        continue
                    # ---- A5: xT = transpose(x[bi]) -> bigB (reuse kT slots) ----
                    xT = [bigB.tile([128, L], F32R, tag=f"b{i}", name=f"xT{bi}_{i}")
                          for i in range(DT)]
                    transpose_into(xT, x_in[bi], BPC + bi)

                    # ---- A6: Q^T = relu(wq^T @ xT + bq) -> qt_s (DRAM) ----
                    for at in range(DT):
                        wqb = wt_p.tile([128, DT, 128], F32R, tag="wt", name=f"wqb{bi}_{at}")
                        nc.sync.dma_start(wqb[:], wq_in[at].bitcast(F32R))
                        for ch in range(4):
                            psq = ps_mm.tile([128, 512], F32, tag="mm", name=f"psq{bi}_{at}_{ch}")
                            for dt in range(DT):
                                nc.tensor.matmul(
                                    psq[:], wqb[:, dt, :],
                                    xT[dt][:, ch * 512:(ch + 1) * 512],
                                    start=(dt == 0), stop=(dt == DT - 1))
                            qst = st_p.tile([128, 512], F32, tag="st", name=f"qst{bi}_{at}_{ch}")
                            nc.scalar.activation(qst[:], psq[:], AF.Relu,
                                                 bias=bq_t[:, at:at + 1])
                            nc.sync.dma_start(
                                qt_s[bi, at * 128:(at + 1) * 128,
                                     ch * 512:(ch + 1) * 512], qst[:])

                    if phase_limit < 6:
                        continue
                    # ---- B: attention ----
                    vB = [bigB.tile([128, 2, D], F32R, tag=f"b{i}", name=f"vB{bi}_{i}")
                          for i in range(DT)]
                    for i in range(DT):
                        for s in range(2):
                            jt = 2 * i + s
                            nc.sync.dma_start(
                                vB[i][:, s, :],
                                v_s[bi, jt * 128:(jt + 1) * 128, :].bitcast(F32R))
                    nmr = sm_p.tile([1, L], F32R, tag="nm", bufs=1, name=f"nmr{bi}")
                    nc.sync.dma_start(nmr[:], nm_in[bi:bi + 1, :].bitcast(F32R))

                    qt_r = qt_s[bi].rearrange("(at p) i -> p at i", p=128)
                    for isub in range(NT):
                        qs = qs_p.tile([128, DT, 128], F32R, tag="qs", name=f"qs{bi}_{isub}")
                        nc.sync.dma_start(
                            qs[:], qt_r[:, :, isub * 128:(isub + 1) * 128].bitcast(F32R))
                        U = u_p.tile([128, L], F32, tag="u", name=f"U{bi}_{isub}")
                        rs4 = sm_p.tile([128, 4], F32, tag="rs4", bufs=2, name=f"rs4{bi}_{isub}")
                        for ch in range(4):
                            pss = ps_mm.tile([128, 512], F32, tag="mm", name=f"pss{bi}_{isub}_{ch}")
                            for at in range(DT):
                                nc.tensor.matmul(
                                    pss[:], qs[:, at, :],
                                    kgT[at][:, ch * 512:(ch + 1) * 512],
                                    start=(at == 0), stop=False)
                            nc.tensor.matmul(
                                pss[:], ones1[:], nmr[:, ch * 512:(ch + 1) * 512],
                                start=False, stop=True)
                            nc.scalar.activation(
                                U[:, ch * 512:(ch + 1) * 512], pss[:], AF.Exp,
                                scale=1.0 / 32.0, accum_out=rs4[:, ch:ch + 1])
                        rs1 = sm_p.tile([128, 1], F32, tag="rs1", bufs=2, name=f"rs1{bi}_{isub}")
                        nc.vector.reduce_sum(rs1[:], rs4[:], axis=mybir.AxisListType.X)
                        rcp = sm_p.tile([128, 1], F32, tag="rcp", bufs=2, name=f"rcp{bi}_{isub}")
                        nc.vector.reciprocal(rcp[:], rs1[:])

                        # transpose RAW U (PE), normalize U in place afterwards
                        if phase_limit < 7:
                            nc.vector.tensor_scalar_mul(U[:], U[:], rcp[:])
                            nc.sync.dma_start(w_out[bi, isub * 128:(isub + 1) * 128, :], U[:])
                            continue
                        WT = wtr_p.tile([128, NT, 128], F32R, tag="wtr", name=f"WT{bi}_{isub}")
                        for jt in range(NT):
                            pst = ps_tr.tile([128, 128], F32, tag="tr", name=f"pstw{bi}_{isub}_{jt}")
                            nc.tensor.transpose(
                                pst[:], U[:, jt * 128:(jt + 1) * 128], ident[:])
                            if jt % 2 == 0:
                                nc.vector.tensor_copy(WT[:, jt, :], pst[:])
                            else:
                                nc.scalar.copy(WT[:, jt, :], pst[:])
                        nc.vector.tensor_scalar_mul(U[:], U[:], rcp[:])
                        nc.sync.dma_start(w_out[bi, isub * 128:(isub + 1) * 128, :], U[:])

                        for ach in range(2):
                            psa = ps_att.tile([128, 512], F32, tag="att", name=f"psa{bi}_{isub}_{ach}")
                            for jt in range(NT):
                                nc.tensor.matmul(
                                    psa[:], WT[:, jt, :],
                                    vB[jt // 2][:, jt % 2, ach * 512:(ach + 1) * 512],
                                    start=(jt == 0), stop=(jt == NT - 1))
                            ast = st_p.tile([128, 512], F32, tag="st", name=f"ast{bi}_{isub}_{ach}")
                            # attention normalization folded into the copy
                            nc.scalar.activation(ast[:], psa[:], AF.Copy, scale=rcp[:])
                            nc.sync.dma_start(
                                att_out[bi, isub * 128:(isub + 1) * 128,
                                        ach * 512:(ach + 1) * 512], ast[:])

            if n_reps == 1:
                body()
            else:
                with tc.For_i(0, n_reps, 1):
                    body()

    nc.compile()
    return nc


_cache = {}


def _get_nc(n_reps: int = 1):
    if n_reps not in _cache:
        _cache[n_reps] = build(n_reps)
    return _cache[n_reps]


def _fmt_lhs(w):
    w4 = np.asarray(w, np.float32).reshape(DT, 128, DT, 128)   # (dt, p, at, c)
    return np.ascontiguousarray(w4.transpose(2, 1, 0, 3))      # (at, p, dt, c)


def _fmt_mov(w):
    w4 = np.asarray(w, np.float32).reshape(DT, 128, 4, 256)    # (dt, p, ach, c)
    return np.ascontiguousarray(w4.transpose(2, 1, 0, 3))      # (ach, p, dt, c)


def make_in_maps(x, y, wq, bq, wk, bk, wv, bv, wg, masks):
    x = np.ascontiguousarray(np.asarray(x, dtype=np.float32))
    y = np.ascontiguousarray(np.asarray(y, dtype=np.float32))
    nmask = np.where(np.asarray(masks), np.float32(0), np.float32(MASK_NEG))
    nmask = np.ascontiguousarray(nmask.astype(np.float32))
    ws = {"wqp": _fmt_lhs(wq), "wkp": _fmt_lhs(wk), "wvp": _fmt_mov(wv),
          "wgp": _fmt_lhs(wg)}
    ws.update({n: np.ascontiguousarray(np.asarray(a, dtype=np.float32))
               for n, a in (("bq", bq), ("bk", bk), ("bv", bv))})
    in_maps = []
    for c in range(N_CORES):
        sl = slice(c * BPC, (c + 1) * BPC)
        in_maps.append({
            "x": np.ascontiguousarray(x[sl]),
            "y": np.ascontiguousarray(y[sl]),
            "nmask": np.ascontiguousarray(nmask[sl]),
            **ws,
        })
    return in_maps


def kernel(x, y, wq, bq, wk, bk, wv, bv, wg, masks):
    nc = _get_nc(1)
    in_maps = make_in_maps(x, y, wq, bq, wk, bk, wv, bv, wg, masks)
    last_exc = None
    for _attempt in range(3):
        try:
            res = run_bass_kernel_spmd(nc, in_maps, list(range(N_CORES)))
            break
        except Exception as e:  # transient NRT device errors: retry
            last_exc = e
    else:
        raise last_exc
    attention = np.concatenate([res.results[c]["att"] for c in range(N_CORES)], axis=0)
    w = np.concatenate([res.results[c]["w"] for c in range(N_CORES)], axis=0)
    return (attention, w)
